# revision 52
# baseline (speedup 1.0000x reference)
"""KAN layer (per-edge tiny MLPs) Trainium2 kernel.

Math (per batch b, output o, input i; H=32 hidden):
  h1 = leaky(x[b,i]*W1[o,i,:] + b1[o,i,:])
  z2 = W2[o,i] @ h1 + b2[o,i]           (per-edge [H,H] matmul)
  h2 = leaky(z2)
  edge = W3[o,i]·h2 + b3[o,i]
  out[b,o] = sum_i (bias_w[o,i]*leaky(x[b,i]) + layer_w[o,i]*edge)

Mapping (8 cores, O sharded, 8 output rows per core), bf16 datapath.
leaky(t) rewritten Pool/DVE-friendly as t' + 99*relu(t') with t' = 0.01*t
(Pool/GPSIMD compiles tensor_scalar + tensor_tensor(add) but not tt(max) or
scalar_tensor_tensor).  Work is split at the *op* level across ACT (1-op
Lrelu evac ~982ns), DVE (tensor_scalar 4x-mode 327ns / tt-add 2x 594ns) and
Pool (flat 853ns/op), targeting ~110us/engine to match PE's 109.7us
(514 bf16 matmuls, cost = out-cols * 0.4167ns).
"""
import sys

sys.path.insert(0, "/opt/trn_rl_repo")

import numpy as np
import ml_dtypes

_B, _I, _O, _H = 1024, 64, 64, 32
_NCORES = 8
_OLOC = _O // _NCORES  # 8 output nodes per core
_ALPHA = 0.01
_NHALF = 512
_BF16 = ml_dtypes.bfloat16

# Pool/GPSIMD cannot read PSUM on real hardware (walrus codegen rejects it).
_POOL_PSUM = False


# --- per-(o,g) routing: balance ACT / DVE / Pool at the op level ---
# Patterns are exclusive (never both on one og) so DVE load stays smooth:
# a split-evac og adds 1519ns to DVE; stacking a DVE h1-add on the same og
# would make a 2.7us burst that stalls the whole ring.
def _h1_route(og):
    # 'split': t',u on DVE; add on Pool.  'dve': all three on DVE.
    # 'act': 1-op Lrelu on ACT, placed right after each split-evac og so it
    # fills the gap ACT would otherwise idle through.
    if og % 10 == 5:
        return "act"
    return "dve" if og % 10 == 9 else "split"


def _evac_route(og):
    # 'act': 1-op Lrelu on ACT.  'split': w,u on DVE, add on Pool.
    return "split" if og % 10 == 4 else "act"


_CACHE = {}


def _build_bass():
    import concourse.bacc as bacc
    import concourse.mybir as mybir
    from concourse.tile import TileContext

    f32 = mybir.dt.float32
    bf16 = mybir.dt.bfloat16
    AF = mybir.ActivationFunctionType
    ALU = mybir.AluOpType

    nc = bacc.Bacc("TRN2", target_bir_lowering=False, debug=False)

    xrep_d = nc.declare_dram_parameter("xrepb", [128, 16 * _B], bf16, isOutput=False)
    xt65_d = nc.declare_dram_parameter("xt65", [65, _B], bf16, isOutput=False)
    # packed [0.01*w1col | 0.01*b1col | b2col | w1col | b1col]
    cols3_d = nc.declare_dram_parameter("cols5", [128, 5 * 128], f32, isOutput=False)
    # [p, og, m] layout so the per-o DMA slice is contiguous per partition
    w2blk_d = nc.declare_dram_parameter("w2blkb", [128, 128, 128], bf16, isOutput=False)
    st4_d = nc.declare_dram_parameter("st4b", [128, 128 * 8], bf16, isOutput=False)
    st5_d = nc.declare_dram_parameter("st5b", [65, 8], bf16, isOutput=False)
    out_d = nc.declare_dram_parameter("out", [8, _B], bf16, isOutput=True)

    with TileContext(nc) as tc:
        with tc.tile_pool(name="consts", bufs=1) as cpool, \
             tc.tile_pool(name="w2", bufs=2) as w2pool, \
             tc.tile_pool(name="h1", bufs=6) as h1pool, \
             tc.tile_pool(name="h2", bufs=4) as h2pool, \
             tc.tile_pool(name="tmp", bufs=10) as tmppool, \
             tc.tile_pool(name="zps", bufs=3, space="PSUM") as zpool, \
             tc.tile_pool(name="ops", bufs=1, space="PSUM") as opool:

            # DMA order = priority order: xrep[g0] + packed h1-chain consts
            # first so DVE starts ~2.5us in, then w2[0] for the first matmul,
            # then early xrep slices; st4/st5 (first needed at mm4/drain) last.
            xrep_t = cpool.tile([128, 16 * _B], bf16)
            nc.sync.dma_start(out=xrep_t[:, 0:_B], in_=xrep_d[:, 0:_B])
            cols3_t = cpool.tile([128, 5 * 128], f32)
            nc.sync.dma_start(out=cols3_t[:], in_=cols3_d[:])
            w2_t0 = w2pool.tile([128, 16 * 128], bf16)
            nc.sync.dma_start(
                out=w2_t0[:],
                in_=w2blk_d[:, 0:16].rearrange("p g m -> p (g m)"))
            nc.sync.dma_start(out=xrep_t[:, _B:2 * _B], in_=xrep_d[:, _B:2 * _B])
            st4_t = cpool.tile([128, 128 * 8], bf16)
            nc.sync.dma_start(out=st4_t[:], in_=st4_d[:])
            nc.sync.dma_start(out=xrep_t[:, 2 * _B:3 * _B],
                              in_=xrep_d[:, 2 * _B:3 * _B])
            # xt65/st5 feed only the drain-time MM5; ride the startup-idle
            # ACT HWDGE queue to keep SP free for xrep slices and w2 blocks
            xt65_t = cpool.tile([65, _B], bf16)
            nc.scalar.dma_start(out=xt65_t[:], in_=xt65_d[:])
            st5_t = cpool.tile([65, 8], bf16)
            nc.scalar.dma_start(out=st5_t[:], in_=st5_d[:])

            lxT_t = cpool.tile([65, _B], bf16)
            nc.scalar.activation(lxT_t[:], xt65_t[:], AF.Lrelu,
                                 bias=0.0, scale=1.0, alpha=_ALPHA)

            outp = opool.tile([8, _B], f32)
            acc_started = [False]

            def emit_mm4(h2_prev, og_prev, last):
                for half in range(2):
                    sl = slice(half * _NHALF, (half + 1) * _NHALF)
                    nc.tensor.matmul(out=outp[:, sl],
                                     lhsT=st4_t[:, og_prev * 8:(og_prev + 1) * 8],
                                     rhs=h2_prev[:, sl],
                                     start=not acc_started[0], stop=last,
                                     skip_group_check=True)
                acc_started[0] = True

            def emit_evac(z2p, ogp):
                h2 = h2pool.tile([128, _B], bf16)
                if _evac_route(ogp) == "split":
                    # h2 = w + 99*relu(w), w = 0.01*(z2 + b2)
                    wv = tmppool.tile([128, _B], bf16)
                    uv = tmppool.tile([128, _B], bf16)
                    nc.vector.tensor_scalar(
                        out=wv[:], in0=z2p[:], scalar1=cols3_t[:, 256 + ogp:257 + ogp],
                        scalar2=_ALPHA, op0=ALU.add, op1=ALU.mult)
                    nc.vector.tensor_scalar(
                        out=uv[:], in0=wv[:], scalar1=0.0, scalar2=99.0,
                        op0=ALU.max, op1=ALU.mult)
                    nc.gpsimd.tensor_tensor(
                        out=h2[:], in0=wv[:], in1=uv[:], op=ALU.add)
                else:
                    nc.scalar.activation(h2[:], z2p[:], AF.Lrelu,
                                         bias=cols3_t[:, 256 + ogp:257 + ogp],
                                         scale=1.0, alpha=_ALPHA)
                return h2

            # software pipeline: mm4 runs 2 ogs behind, evac 1 og behind, so
            # every emitted instruction's inputs are already (nearly) ready.
            pend_evacs = []  # [(z2, og), ...], evac runs 1 og behind
            pend_mm4 = None  # (h2, og)

            def flush_evacs(cur_og):
                nonlocal pend_mm4
                keep = []
                for z2p, ogp in pend_evacs:
                    if cur_og - ogp >= 1:
                        if pend_mm4 is not None:
                            emit_mm4(*pend_mm4, last=False)
                        h2p = emit_evac(z2p, ogp)
                        pend_mm4 = (h2p, ogp)
                    else:
                        keep.append((z2p, ogp))
                pend_evacs[:] = keep

            for o in range(_OLOC):
                if o == 0:
                    w2_t = w2_t0
                else:
                    w2_t = w2pool.tile([128, 16 * 128], bf16)
                    nc.sync.dma_start(
                        out=w2_t[:],
                        in_=w2blk_d[:, o * 16:(o + 1) * 16].rearrange(
                            "p g m -> p (g m)"),
                    )
                for g in range(16):
                    og = o * 16 + g
                    if o == 0 and 2 <= g < 15:
                        # prefetch next xrep slice during the first o pass
                        gn = g + 1
                        nc.sync.dma_start(out=xrep_t[:, gn * _B:(gn + 1) * _B],
                                          in_=xrep_d[:, gn * _B:(gn + 1) * _B])
                    xg = xrep_t[:, g * _B:(g + 1) * _B]
                    # h1 = t' + 99*relu(t');  t' = 0.01*(W1*x + b1)
                    h1 = h1pool.tile([128, _B], bf16)
                    route = _h1_route(og)
                    if route == "act":
                        nc.scalar.activation(
                            h1[:], xg, AF.Lrelu,
                            bias=cols3_t[:, 512 + og:513 + og],
                            scale=cols3_t[:, 384 + og:385 + og], alpha=_ALPHA)
                    else:
                        tp = tmppool.tile([128, _B], bf16)
                        up = tmppool.tile([128, _B], bf16)
                        nc.vector.tensor_scalar(
                            out=tp[:], in0=xg, scalar1=cols3_t[:, og:og + 1],
                            scalar2=cols3_t[:, 128 + og:129 + og],
                            op0=ALU.mult, op1=ALU.add)
                        nc.vector.tensor_scalar(
                            out=up[:], in0=tp[:], scalar1=0.0, scalar2=99.0,
                            op0=ALU.max, op1=ALU.mult)
                        if route == "dve":
                            nc.vector.tensor_tensor(
                                out=h1[:], in0=tp[:], in1=up[:], op=ALU.add)
                        else:
                            nc.gpsimd.tensor_tensor(
                                out=h1[:], in0=tp[:], in1=up[:], op=ALU.add)
                    z2 = zpool.tile([128, _B], f32)
                    for half in range(2):
                        sl = slice(half * _NHALF, (half + 1) * _NHALF)
                        nc.tensor.matmul(out=z2[:, sl],
                                         lhsT=w2_t[:, g * 128:(g + 1) * 128],
                                         rhs=h1[:, sl], start=True, stop=True)
                    flush_evacs(og)
                    pend_evacs.append((z2, og))
            # drain: flush all but the final og, then MM5 (bias_w*leaky(x) +
            # consts) joins the accumulation so PE's output group never waits
            # on lx/table-load. The last og's evac/mm4/copy/DMA pipeline per
            # PSUM-bank half so half 0 streams out while half 1 is still
            # being evacuated.
            while len(pend_evacs) > 1:
                z2p, ogp = pend_evacs.pop(0)
                if pend_mm4 is not None:
                    emit_mm4(*pend_mm4, last=False)
                h2p = emit_evac(z2p, ogp)
                pend_mm4 = (h2p, ogp)
            emit_mm4(*pend_mm4, last=False)
            for half in range(2):
                sl = slice(half * _NHALF, (half + 1) * _NHALF)
                nc.tensor.matmul(out=outp[:, sl], lhsT=st5_t[:], rhs=lxT_t[:, sl],
                                 start=False, stop=False, skip_group_check=True)
            z2p, ogp = pend_evacs.pop(0)
            h2p = h2pool.tile([128, _B], bf16)
            outs = cpool.tile([8, _B], bf16)
            for half in range(2):
                sl = slice(half * _NHALF, (half + 1) * _NHALF)
                nc.scalar.activation(h2p[:, sl], z2p[:, sl], AF.Lrelu,
                                     bias=cols3_t[:, 256 + ogp:257 + ogp],
                                     scale=1.0, alpha=_ALPHA)
                nc.tensor.matmul(out=outp[:, sl],
                                 lhsT=st4_t[:, ogp * 8:(ogp + 1) * 8],
                                 rhs=h2p[:, sl], start=False, stop=True,
                                 skip_group_check=True)
                if half == 0:
                    nc.vector.tensor_copy(outs[:, sl], outp[:, sl])
                    nc.sync.dma_start(out=out_d[:, sl], in_=outs[:, sl])
                else:
                    nc.scalar.copy(outs[:, sl], outp[:, sl])
                    nc.scalar.dma_start(out=out_d[:, sl], in_=outs[:, sl])

    nc.finalize()
    return nc


def _prepare_inputs(x, W1, b1, W2, b2, W3, b3, layer_w, bias_w):
    f = np.float32
    x = np.asarray(x, f)
    xT = np.ascontiguousarray(x.T)                      # [I, B]
    # xrepb[32j+h, g*B+b] = x[4g+j, b]
    xq = xT.reshape(16, 4, _B).transpose(1, 0, 2)       # [j, g, b]
    xrepb = np.ascontiguousarray(
        np.repeat(xq, _H, axis=0).reshape(128, 16 * _B)).astype(_BF16)
    xt65 = np.concatenate([xT, np.ones((1, _B), f)], 0).astype(_BF16)  # [65, B]

    v = (np.asarray(layer_w, f)[:, :, None] * np.asarray(W3, f))  # [O,I,H]
    w2f = np.asarray(W2, f)

    in_maps = []
    for c in range(_NCORES):
        sl = slice(c * _OLOC, (c + 1) * _OLOC)
        W1c, b1c, b2c = W1[sl], b1[sl], b2[sl]          # [8,64,H]
        W2c = w2f[sl]                                   # [8,64,H,H]
        vc = v[sl]
        lwc, bwc, b3c = layer_w[sl], bias_w[sl], b3[sl]

        # [o, g, j, h] -> partition 32j+h, col o*16+g
        def cols(a):  # a [8, 64, 32] -> [128, 128]
            a = np.asarray(a, f).reshape(_OLOC, 16, 4, _H)
            return np.ascontiguousarray(
                a.transpose(2, 3, 0, 1).reshape(128, 128))

        # packed [0.01*w1 | 0.01*b1 | b2 | w1 | b1] (scaled for the DVE/Pool
        # relu-decomposition; raw for the 1-op ACT leaky route)
        w1c_, b1c_ = cols(W1c), cols(b1c)
        cols3 = np.ascontiguousarray(np.concatenate(
            [w1c_ * _ALPHA, b1c_ * _ALPHA, cols(b2c), w1c_, b1c_], axis=1))

        # block-diagonal lhsT: blk[og][32j+h, 32j+k] = W2[o,4g+j,k,h]
        W2t = W2c.transpose(0, 1, 3, 2).reshape(_OLOC, 16, 4, _H, _H)
        w2blk = np.zeros((_OLOC, 16, 128, 128), f)
        for j in range(4):
            w2blk[:, :, 32 * j:32 * j + 32, 32 * j:32 * j + 32] = W2t[:, :, j]
        # -> [p, og, m] layout, bf16
        w2blkb = np.ascontiguousarray(
            w2blk.reshape(128, 128, 128).transpose(1, 0, 2)).astype(_BF16)

        # st4[og][32j+k, o] = v[o,4g+j,k]
        def stack8b(a):
            a = np.asarray(a, f).reshape(_OLOC, 16, 4 * _H)
            out = np.zeros((128, _OLOC * 16, _OLOC), f)
            for o in range(_OLOC):
                for g in range(16):
                    out[:, o * 16 + g, o] = a[o, g]
            return np.ascontiguousarray(out.reshape(128, 128 * _OLOC))

        st4b = stack8b(vc).astype(_BF16)

        st5 = np.zeros((65, _OLOC), f)
        st5[:_I, :] = np.asarray(bwc, f).T              # bias_w[o,i] at row i
        const = (np.asarray(lwc, f) * np.asarray(b3c, f)).sum(1)
        st5[_I, :] = const
        st5b = st5.astype(_BF16)

        in_maps.append({
            "xrepb": xrepb, "xt65": xt65,
            "cols5": cols3,
            "w2blkb": w2blkb, "st4b": st4b, "st5b": st5b,
        })
    return in_maps


def kernel(x, W1, b1, W2, b2, W3, b3, layer_w, bias_w):
    from concourse.bass_utils import run_bass_kernel_spmd

    if "nc" not in _CACHE:
        _CACHE["nc"] = _build_bass()
    nc = _CACHE["nc"]

    in_maps = _prepare_inputs(x, W1, b1, W2, b2, W3, b3, layer_w, bias_w)
    res = run_bass_kernel_spmd(nc, in_maps, list(range(_NCORES))).results

    out = np.empty((_B, _O), np.float32)
    for c in range(_NCORES):
        out[:, c * _OLOC:(c + 1) * _OLOC] = np.asarray(
            res[c]["out"], np.float32).T
    return out


if __name__ == "__main__":
    # quick self-check against a numpy reference
    rng = np.random.default_rng(0)
    f = np.float32
    inputs = {
        "x": rng.standard_normal((_B, _I), f),
        "W1": rng.uniform(-1, 1, (_O, _I, _H)).astype(f),
        "b1": rng.uniform(-1, 1, (_O, _I, _H)).astype(f),
        "W2": rng.uniform(-0.2, 0.2, (_O, _I, _H, _H)).astype(f),
        "b2": rng.uniform(-0.2, 0.2, (_O, _I, _H)).astype(f),
        "W3": rng.uniform(-0.2, 0.2, (_O, _I, _H)).astype(f),
        "b3": rng.uniform(-0.2, 0.2, (_O, _I)).astype(f),
        "layer_w": np.ones((_O, _I), f),
        "bias_w": rng.uniform(-0.1, 0.1, (_O, _I)).astype(f),
    }

    def leaky(a):
        return np.where(a >= 0, a, _ALPHA * a)

    def ref(x, W1, b1, W2, b2, W3, b3, layer_w, bias_w):
        h1 = leaky(x[:, None, :, None] * W1 + b1)
        h2 = leaky(np.einsum("boih,oikh->boik", h1, W2) + b2)
        edge = np.einsum("boih,oih->boi", h2, W3) + b3
        edge = bias_w * leaky(x)[:, None, :] + layer_w * edge
        return edge.sum(axis=2)

    expected = ref(**{k: np.asarray(val, np.float64) for k, val in inputs.items()})
    actual = kernel(**inputs)
    err = np.abs(actual - expected).max() / np.abs(expected).max()
    print("rel err:", err)


# revision 53
# speedup vs baseline: 1.0356x; 1.0356x over previous
"""KAN layer (per-edge tiny MLPs) Trainium2 kernel.

Math (per batch b, output o, input i; H=32 hidden):
  h1 = leaky(x[b,i]*W1[o,i,:] + b1[o,i,:])
  z2 = W2[o,i] @ h1 + b2[o,i]           (per-edge [H,H] matmul)
  h2 = leaky(z2)
  edge = W3[o,i]·h2 + b3[o,i]
  out[b,o] = sum_i (bias_w[o,i]*leaky(x[b,i]) + layer_w[o,i]*edge)

Mapping (8 cores, O sharded, 8 output rows per core), bf16 datapath.
leaky(t) rewritten Pool/DVE-friendly as t' + 99*relu(t') with t' = 0.01*t
(Pool/GPSIMD compiles tensor_scalar + tensor_tensor(add) but not tt(max) or
scalar_tensor_tensor).  Work is split at the *op* level across ACT (1-op
Lrelu evac ~982ns), DVE (tensor_scalar 4x-mode 327ns / tt-add 2x 594ns) and
Pool (flat 853ns/op), targeting ~110us/engine to match PE's 109.7us
(514 bf16 matmuls, cost = out-cols * 0.4167ns).
"""
import sys

sys.path.insert(0, "/opt/trn_rl_repo")

import numpy as np
import ml_dtypes

_B, _I, _O, _H = 1024, 64, 64, 32
_NCORES = 8
_OLOC = _O // _NCORES  # 8 output nodes per core
_ALPHA = 0.01
_NHALF = 512
_BF16 = ml_dtypes.bfloat16

# Pool/GPSIMD cannot read PSUM on real hardware (walrus codegen rejects it).
_POOL_PSUM = False


# --- per-(o,g) routing: balance ACT / DVE / Pool at the op level ---
# Patterns are exclusive (never both on one og) so DVE load stays smooth:
# a split-evac og adds 1519ns to DVE; stacking a DVE h1-add on the same og
# would make a 2.7us burst that stalls the whole ring.
def _h1_route(og):
    # 'split': t',u on DVE; add on Pool.  'dve': all three on DVE.
    # 'act': 1-op Lrelu on ACT, placed right after each split-evac og so it
    # fills the gap ACT would otherwise idle through.
    if og % 10 == 5:
        return "act"
    return "dve" if og % 10 == 9 else "split"


def _evac_route(og):
    # 'act': 1-op Lrelu on ACT.  'split': w,u on DVE, add on Pool.
    return "split" if og % 10 == 4 else "act"


_CACHE = {}


def _build_bass():
    import concourse.bacc as bacc
    import concourse.mybir as mybir
    from concourse.tile import TileContext

    f32 = mybir.dt.float32
    bf16 = mybir.dt.bfloat16
    AF = mybir.ActivationFunctionType
    ALU = mybir.AluOpType

    nc = bacc.Bacc("TRN2", target_bir_lowering=False, debug=False)

    xrep_d = nc.declare_dram_parameter("xrepb", [128, 16 * _B], bf16, isOutput=False)
    xt65_d = nc.declare_dram_parameter("xt65", [65, _B], bf16, isOutput=False)
    # packed [0.01*w1col | 0.01*b1col | b2col | w1col | b1col]
    cols3_d = nc.declare_dram_parameter("cols5", [128, 5 * 128], f32, isOutput=False)
    # [p, og, m] layout so the per-o DMA slice is contiguous per partition
    w2blk_d = nc.declare_dram_parameter("w2blkb", [128, 128, 128], bf16, isOutput=False)
    st4_d = nc.declare_dram_parameter("st4b", [128, 128 * 8], bf16, isOutput=False)
    st5_d = nc.declare_dram_parameter("st5b", [65, 8], bf16, isOutput=False)
    out_d = nc.declare_dram_parameter("out", [8, _B], bf16, isOutput=True)

    with TileContext(nc) as tc:
        with tc.tile_pool(name="consts", bufs=1) as cpool, \
             tc.tile_pool(name="w2", bufs=2) as w2pool, \
             tc.tile_pool(name="h1", bufs=6) as h1pool, \
             tc.tile_pool(name="h2", bufs=4) as h2pool, \
             tc.tile_pool(name="tmp", bufs=10) as tmppool, \
             tc.tile_pool(name="zps", bufs=3, space="PSUM") as zpool, \
             tc.tile_pool(name="ops", bufs=1, space="PSUM") as opool:

            # DMA order = priority order: xrep[g0] + packed h1-chain consts
            # first so DVE starts ~2.5us in, then w2[0] for the first matmul,
            # then early xrep slices; st4/st5 (first needed at mm4/drain) last.
            xrep_t = cpool.tile([128, 16 * _B], bf16)
            nc.sync.dma_start(out=xrep_t[:, 0:_B], in_=xrep_d[:, 0:_B])
            cols3_t = cpool.tile([128, 5 * 128], f32)
            nc.sync.dma_start(out=cols3_t[:], in_=cols3_d[:])
            w2_t0 = w2pool.tile([128, 16 * 128], bf16)
            nc.sync.dma_start(
                out=w2_t0[:],
                in_=w2blk_d[:, 0:16].rearrange("p g m -> p (g m)"))
            nc.sync.dma_start(out=xrep_t[:, _B:2 * _B], in_=xrep_d[:, _B:2 * _B])
            st4_t = cpool.tile([128, 128 * 8], bf16)
            nc.sync.dma_start(out=st4_t[:], in_=st4_d[:])
            nc.sync.dma_start(out=xrep_t[:, 2 * _B:3 * _B],
                              in_=xrep_d[:, 2 * _B:3 * _B])
            # xt65/st5 feed only the drain-time MM5; ride the startup-idle
            # ACT HWDGE queue to keep SP free for xrep slices and w2 blocks
            xt65_t = cpool.tile([65, _B], bf16)
            nc.scalar.dma_start(out=xt65_t[:], in_=xt65_d[:])
            st5_t = cpool.tile([65, 8], bf16)
            nc.scalar.dma_start(out=st5_t[:], in_=st5_d[:])

            lxT_t = cpool.tile([65, _B], bf16)
            nc.scalar.activation(lxT_t[:], xt65_t[:], AF.Lrelu,
                                 bias=0.0, scale=1.0, alpha=_ALPHA)

            outp = opool.tile([8, _B], f32)
            acc_started = [False]

            def emit_mm4(h2_prev, og_prev, last):
                for half in range(2):
                    sl = slice(half * _NHALF, (half + 1) * _NHALF)
                    nc.tensor.matmul(out=outp[:, sl],
                                     lhsT=st4_t[:, og_prev * 8:(og_prev + 1) * 8],
                                     rhs=h2_prev[:, sl],
                                     start=not acc_started[0], stop=last,
                                     skip_group_check=True)
                acc_started[0] = True

            def emit_evac(z2p, ogp):
                h2 = h2pool.tile([128, _B], bf16)
                if _evac_route(ogp) == "split":
                    # h2 = w + 99*relu(w), w = 0.01*(z2 + b2)
                    wv = tmppool.tile([128, _B], bf16)
                    uv = tmppool.tile([128, _B], bf16)
                    nc.vector.tensor_scalar(
                        out=wv[:], in0=z2p[:], scalar1=cols3_t[:, 256 + ogp:257 + ogp],
                        scalar2=_ALPHA, op0=ALU.add, op1=ALU.mult)
                    nc.vector.tensor_scalar(
                        out=uv[:], in0=wv[:], scalar1=0.0, scalar2=99.0,
                        op0=ALU.max, op1=ALU.mult)
                    nc.gpsimd.tensor_tensor(
                        out=h2[:], in0=wv[:], in1=uv[:], op=ALU.add)
                else:
                    nc.scalar.activation(h2[:], z2p[:], AF.Lrelu,
                                         bias=cols3_t[:, 256 + ogp:257 + ogp],
                                         scale=1.0, alpha=_ALPHA)
                return h2

            # software pipeline: mm4 runs 2 ogs behind, evac 1 og behind, so
            # every emitted instruction's inputs are already (nearly) ready.
            pend_evacs = []  # [(z2, og), ...], evac runs 1 og behind
            pend_mm4 = None  # (h2, og)
            h1_future = {}   # og -> pre-built ACT-route h1 tile

            def flush_evacs(cur_og):
                nonlocal pend_mm4
                keep = []
                for z2p, ogp in pend_evacs:
                    if cur_og - ogp >= 1:
                        if pend_mm4 is not None:
                            emit_mm4(*pend_mm4, last=False)
                        h2p = emit_evac(z2p, ogp)
                        pend_mm4 = (h2p, ogp)
                    else:
                        keep.append((z2p, ogp))
                pend_evacs[:] = keep

            for o in range(_OLOC):
                if o == 0:
                    w2_t = w2_t0
                else:
                    w2_t = w2pool.tile([128, 16 * 128], bf16)
                    nc.sync.dma_start(
                        out=w2_t[:],
                        in_=w2blk_d[:, o * 16:(o + 1) * 16].rearrange(
                            "p g m -> p (g m)"),
                    )
                for g in range(16):
                    og = o * 16 + g
                    if o == 0 and 2 <= g < 15:
                        # prefetch next xrep slice during the first o pass
                        gn = g + 1
                        nc.sync.dma_start(out=xrep_t[:, gn * _B:(gn + 1) * _B],
                                          in_=xrep_d[:, gn * _B:(gn + 1) * _B])
                    # ACT-route h1s are emitted 3 ogs ahead of their consumer
                    # so they drain through ACT's in-order queue with slack
                    # (they only need xrep+consts, so they fill ACT's
                    # split-evac gaps instead of blocking MM2 just-in-time).
                    tgt = og + 3
                    if tgt < 128 and _h1_route(tgt) == "act":
                        ha = h1pool.tile([128, _B], bf16)
                        gt = tgt % 16
                        nc.scalar.activation(
                            ha[:], xrep_t[:, gt * _B:(gt + 1) * _B], AF.Lrelu,
                            bias=cols3_t[:, 512 + tgt:513 + tgt],
                            scale=cols3_t[:, 384 + tgt:385 + tgt], alpha=_ALPHA)
                        h1_future[tgt] = ha
                    xg = xrep_t[:, g * _B:(g + 1) * _B]
                    # h1 = t' + 99*relu(t');  t' = 0.01*(W1*x + b1)
                    route = _h1_route(og)
                    if route == "act":
                        h1 = h1_future.pop(og)
                    else:
                        h1 = h1pool.tile([128, _B], bf16)
                        tp = tmppool.tile([128, _B], bf16)
                        up = tmppool.tile([128, _B], bf16)
                        nc.vector.tensor_scalar(
                            out=tp[:], in0=xg, scalar1=cols3_t[:, og:og + 1],
                            scalar2=cols3_t[:, 128 + og:129 + og],
                            op0=ALU.mult, op1=ALU.add)
                        nc.vector.tensor_scalar(
                            out=up[:], in0=tp[:], scalar1=0.0, scalar2=99.0,
                            op0=ALU.max, op1=ALU.mult)
                        if route == "dve":
                            nc.vector.tensor_tensor(
                                out=h1[:], in0=tp[:], in1=up[:], op=ALU.add)
                        else:
                            nc.gpsimd.tensor_tensor(
                                out=h1[:], in0=tp[:], in1=up[:], op=ALU.add)
                    z2 = zpool.tile([128, _B], f32)
                    for half in range(2):
                        sl = slice(half * _NHALF, (half + 1) * _NHALF)
                        nc.tensor.matmul(out=z2[:, sl],
                                         lhsT=w2_t[:, g * 128:(g + 1) * 128],
                                         rhs=h1[:, sl], start=True, stop=True)
                    flush_evacs(og)
                    pend_evacs.append((z2, og))
            # drain: flush all but the final og, then MM5 (bias_w*leaky(x) +
            # consts) joins the accumulation so PE's output group never waits
            # on lx/table-load. The last og's evac/mm4/copy/DMA pipeline per
            # PSUM-bank half so half 0 streams out while half 1 is still
            # being evacuated.
            while len(pend_evacs) > 1:
                z2p, ogp = pend_evacs.pop(0)
                if pend_mm4 is not None:
                    emit_mm4(*pend_mm4, last=False)
                h2p = emit_evac(z2p, ogp)
                pend_mm4 = (h2p, ogp)
            emit_mm4(*pend_mm4, last=False)
            for half in range(2):
                sl = slice(half * _NHALF, (half + 1) * _NHALF)
                nc.tensor.matmul(out=outp[:, sl], lhsT=st5_t[:], rhs=lxT_t[:, sl],
                                 start=False, stop=False, skip_group_check=True)
            z2p, ogp = pend_evacs.pop(0)
            h2p = h2pool.tile([128, _B], bf16)
            outs = cpool.tile([8, _B], bf16)
            for half in range(2):
                sl = slice(half * _NHALF, (half + 1) * _NHALF)
                nc.scalar.activation(h2p[:, sl], z2p[:, sl], AF.Lrelu,
                                     bias=cols3_t[:, 256 + ogp:257 + ogp],
                                     scale=1.0, alpha=_ALPHA)
                nc.tensor.matmul(out=outp[:, sl],
                                 lhsT=st4_t[:, ogp * 8:(ogp + 1) * 8],
                                 rhs=h2p[:, sl], start=False, stop=True,
                                 skip_group_check=True)
                if half == 0:
                    nc.vector.tensor_copy(outs[:, sl], outp[:, sl])
                    nc.sync.dma_start(out=out_d[:, sl], in_=outs[:, sl])
                else:
                    nc.scalar.copy(outs[:, sl], outp[:, sl])
                    nc.scalar.dma_start(out=out_d[:, sl], in_=outs[:, sl])

    nc.finalize()
    return nc


def _prepare_inputs(x, W1, b1, W2, b2, W3, b3, layer_w, bias_w):
    f = np.float32
    x = np.asarray(x, f)
    xT = np.ascontiguousarray(x.T)                      # [I, B]
    # xrepb[32j+h, g*B+b] = x[4g+j, b]
    xq = xT.reshape(16, 4, _B).transpose(1, 0, 2)       # [j, g, b]
    xrepb = np.ascontiguousarray(
        np.repeat(xq, _H, axis=0).reshape(128, 16 * _B)).astype(_BF16)
    xt65 = np.concatenate([xT, np.ones((1, _B), f)], 0).astype(_BF16)  # [65, B]

    v = (np.asarray(layer_w, f)[:, :, None] * np.asarray(W3, f))  # [O,I,H]
    w2f = np.asarray(W2, f)

    in_maps = []
    for c in range(_NCORES):
        sl = slice(c * _OLOC, (c + 1) * _OLOC)
        W1c, b1c, b2c = W1[sl], b1[sl], b2[sl]          # [8,64,H]
        W2c = w2f[sl]                                   # [8,64,H,H]
        vc = v[sl]
        lwc, bwc, b3c = layer_w[sl], bias_w[sl], b3[sl]

        # [o, g, j, h] -> partition 32j+h, col o*16+g
        def cols(a):  # a [8, 64, 32] -> [128, 128]
            a = np.asarray(a, f).reshape(_OLOC, 16, 4, _H)
            return np.ascontiguousarray(
                a.transpose(2, 3, 0, 1).reshape(128, 128))

        # packed [0.01*w1 | 0.01*b1 | b2 | w1 | b1] (scaled for the DVE/Pool
        # relu-decomposition; raw for the 1-op ACT leaky route)
        w1c_, b1c_ = cols(W1c), cols(b1c)
        cols3 = np.ascontiguousarray(np.concatenate(
            [w1c_ * _ALPHA, b1c_ * _ALPHA, cols(b2c), w1c_, b1c_], axis=1))

        # block-diagonal lhsT: blk[og][32j+h, 32j+k] = W2[o,4g+j,k,h]
        W2t = W2c.transpose(0, 1, 3, 2).reshape(_OLOC, 16, 4, _H, _H)
        w2blk = np.zeros((_OLOC, 16, 128, 128), f)
        for j in range(4):
            w2blk[:, :, 32 * j:32 * j + 32, 32 * j:32 * j + 32] = W2t[:, :, j]
        # -> [p, og, m] layout, bf16
        w2blkb = np.ascontiguousarray(
            w2blk.reshape(128, 128, 128).transpose(1, 0, 2)).astype(_BF16)

        # st4[og][32j+k, o] = v[o,4g+j,k]
        def stack8b(a):
            a = np.asarray(a, f).reshape(_OLOC, 16, 4 * _H)
            out = np.zeros((128, _OLOC * 16, _OLOC), f)
            for o in range(_OLOC):
                for g in range(16):
                    out[:, o * 16 + g, o] = a[o, g]
            return np.ascontiguousarray(out.reshape(128, 128 * _OLOC))

        st4b = stack8b(vc).astype(_BF16)

        st5 = np.zeros((65, _OLOC), f)
        st5[:_I, :] = np.asarray(bwc, f).T              # bias_w[o,i] at row i
        const = (np.asarray(lwc, f) * np.asarray(b3c, f)).sum(1)
        st5[_I, :] = const
        st5b = st5.astype(_BF16)

        in_maps.append({
            "xrepb": xrepb, "xt65": xt65,
            "cols5": cols3,
            "w2blkb": w2blkb, "st4b": st4b, "st5b": st5b,
        })
    return in_maps


def kernel(x, W1, b1, W2, b2, W3, b3, layer_w, bias_w):
    from concourse.bass_utils import run_bass_kernel_spmd

    if "nc" not in _CACHE:
        _CACHE["nc"] = _build_bass()
    nc = _CACHE["nc"]

    in_maps = _prepare_inputs(x, W1, b1, W2, b2, W3, b3, layer_w, bias_w)
    res = run_bass_kernel_spmd(nc, in_maps, list(range(_NCORES))).results

    out = np.empty((_B, _O), np.float32)
    for c in range(_NCORES):
        out[:, c * _OLOC:(c + 1) * _OLOC] = np.asarray(
            res[c]["out"], np.float32).T
    return out


if __name__ == "__main__":
    # quick self-check against a numpy reference
    rng = np.random.default_rng(0)
    f = np.float32
    inputs = {
        "x": rng.standard_normal((_B, _I), f),
        "W1": rng.uniform(-1, 1, (_O, _I, _H)).astype(f),
        "b1": rng.uniform(-1, 1, (_O, _I, _H)).astype(f),
        "W2": rng.uniform(-0.2, 0.2, (_O, _I, _H, _H)).astype(f),
        "b2": rng.uniform(-0.2, 0.2, (_O, _I, _H)).astype(f),
        "W3": rng.uniform(-0.2, 0.2, (_O, _I, _H)).astype(f),
        "b3": rng.uniform(-0.2, 0.2, (_O, _I)).astype(f),
        "layer_w": np.ones((_O, _I), f),
        "bias_w": rng.uniform(-0.1, 0.1, (_O, _I)).astype(f),
    }

    def leaky(a):
        return np.where(a >= 0, a, _ALPHA * a)

    def ref(x, W1, b1, W2, b2, W3, b3, layer_w, bias_w):
        h1 = leaky(x[:, None, :, None] * W1 + b1)
        h2 = leaky(np.einsum("boih,oikh->boik", h1, W2) + b2)
        edge = np.einsum("boih,oih->boi", h2, W3) + b3
        edge = bias_w * leaky(x)[:, None, :] + layer_w * edge
        return edge.sum(axis=2)

    expected = ref(**{k: np.asarray(val, np.float64) for k, val in inputs.items()})
    actual = kernel(**inputs)
    err = np.abs(actual - expected).max() / np.abs(expected).max()
    print("rel err:", err)


# revision 54
# speedup vs baseline: 1.0808x; 1.0437x over previous
"""KAN layer (per-edge tiny MLPs) Trainium2 kernel.

Math (per batch b, output o, input i; H=32 hidden):
  h1 = leaky(x[b,i]*W1[o,i,:] + b1[o,i,:])
  z2 = W2[o,i] @ h1 + b2[o,i]           (per-edge [H,H] matmul)
  h2 = leaky(z2)
  edge = W3[o,i]·h2 + b3[o,i]
  out[b,o] = sum_i (bias_w[o,i]*leaky(x[b,i]) + layer_w[o,i]*edge)

Mapping (8 cores, O sharded, 8 output rows per core), bf16 datapath.
leaky(t) rewritten Pool/DVE-friendly as t' + 99*relu(t') with t' = 0.01*t
(Pool/GPSIMD compiles tensor_scalar + tensor_tensor(add) but not tt(max) or
scalar_tensor_tensor).  Work is split at the *op* level across ACT (1-op
Lrelu evac ~982ns), DVE (tensor_scalar 4x-mode 327ns / tt-add 2x 594ns) and
Pool (flat 853ns/op), targeting ~110us/engine to match PE's 109.7us
(514 bf16 matmuls, cost = out-cols * 0.4167ns).
"""
import sys

sys.path.insert(0, "/opt/trn_rl_repo")

import numpy as np
import ml_dtypes

_B, _I, _O, _H = 1024, 64, 64, 32
_NCORES = 8
_OLOC = _O // _NCORES  # 8 output nodes per core
_ALPHA = 0.01
_NHALF = 512
_BF16 = ml_dtypes.bfloat16

# Pool/GPSIMD cannot read PSUM on real hardware (walrus codegen rejects it).
_POOL_PSUM = False


# --- per-(o,g) routing: balance ACT / DVE / Pool at the op level ---
# Patterns are exclusive (never both on one og) so DVE load stays smooth:
# a split-evac og adds 1519ns to DVE; stacking a DVE h1-add on the same og
# would make a 2.7us burst that stalls the whole ring.
def _h1_route(og):
    # 'split': t',u on DVE; add on Pool.  'dve': all three on DVE.
    return "dve" if og % 10 == 9 else "split"


def _evac_route(og):
    # 'act': 1-op Lrelu on ACT.  'split': w,u on DVE, add on Pool.
    return "split" if og % 10 == 4 else "act"


_CACHE = {}


def _build_bass():
    import concourse.bacc as bacc
    import concourse.mybir as mybir
    from concourse.tile import TileContext

    f32 = mybir.dt.float32
    bf16 = mybir.dt.bfloat16
    AF = mybir.ActivationFunctionType
    ALU = mybir.AluOpType

    nc = bacc.Bacc("TRN2", target_bir_lowering=False, debug=False)

    xrep_d = nc.declare_dram_parameter("xrepb", [128, 16 * _B], bf16, isOutput=False)
    xt65_d = nc.declare_dram_parameter("xt65", [65, _B], bf16, isOutput=False)
    # packed [w1col | b1col | b2col], w1col/b1col pre-scaled by 0.01 on host
    cols3_d = nc.declare_dram_parameter("cols3", [128, 3 * 128], f32, isOutput=False)
    # [p, og, m] layout so the per-o DMA slice is contiguous per partition
    w2blk_d = nc.declare_dram_parameter("w2blkb", [128, 128, 128], bf16, isOutput=False)
    st4_d = nc.declare_dram_parameter("st4b", [128, 128 * 8], bf16, isOutput=False)
    st5_d = nc.declare_dram_parameter("st5b", [65, 8], bf16, isOutput=False)
    out_d = nc.declare_dram_parameter("out", [8, _B], bf16, isOutput=True)

    with TileContext(nc) as tc:
        with tc.tile_pool(name="consts", bufs=1) as cpool, \
             tc.tile_pool(name="w2", bufs=2) as w2pool, \
             tc.tile_pool(name="h1", bufs=6) as h1pool, \
             tc.tile_pool(name="h2", bufs=4) as h2pool, \
             tc.tile_pool(name="tmp", bufs=10) as tmppool, \
             tc.tile_pool(name="zps", bufs=3, space="PSUM") as zpool, \
             tc.tile_pool(name="ops", bufs=1, space="PSUM") as opool:

            # DMA order = priority order: xrep[g0] + packed h1-chain consts
            # first so DVE starts ~2.5us in, then w2[0] for the first matmul,
            # then early xrep slices; st4/st5 (first needed at mm4/drain) last.
            xrep_t = cpool.tile([128, 16 * _B], bf16)
            nc.sync.dma_start(out=xrep_t[:, 0:_B], in_=xrep_d[:, 0:_B])
            cols3_t = cpool.tile([128, 3 * 128], f32)
            nc.sync.dma_start(out=cols3_t[:], in_=cols3_d[:])
            w2_t0 = w2pool.tile([128, 16 * 128], bf16)
            nc.sync.dma_start(
                out=w2_t0[:],
                in_=w2blk_d[:, 0:16].rearrange("p g m -> p (g m)"))
            nc.sync.dma_start(out=xrep_t[:, _B:2 * _B], in_=xrep_d[:, _B:2 * _B])
            st4_t = cpool.tile([128, 128 * 8], bf16)
            nc.sync.dma_start(out=st4_t[:], in_=st4_d[:])
            nc.sync.dma_start(out=xrep_t[:, 2 * _B:3 * _B],
                              in_=xrep_d[:, 2 * _B:3 * _B])
            # xt65/st5 feed only the drain-time MM5; ride the startup-idle
            # ACT HWDGE queue to keep SP free for xrep slices and w2 blocks
            xt65_t = cpool.tile([65, _B], bf16)
            nc.scalar.dma_start(out=xt65_t[:], in_=xt65_d[:])
            st5_t = cpool.tile([65, 8], bf16)
            nc.scalar.dma_start(out=st5_t[:], in_=st5_d[:])

            lxT_t = cpool.tile([65, _B], bf16)
            nc.scalar.activation(lxT_t[:], xt65_t[:], AF.Lrelu,
                                 bias=0.0, scale=1.0, alpha=_ALPHA)

            outp = opool.tile([8, _B], f32)
            acc_started = [False]

            def emit_mm4(h2_prev, og_prev, last):
                for half in range(2):
                    sl = slice(half * _NHALF, (half + 1) * _NHALF)
                    nc.tensor.matmul(out=outp[:, sl],
                                     lhsT=st4_t[:, og_prev * 8:(og_prev + 1) * 8],
                                     rhs=h2_prev[:, sl],
                                     start=not acc_started[0], stop=last,
                                     skip_group_check=True)
                acc_started[0] = True

            def emit_evac(z2p, ogp):
                h2 = h2pool.tile([128, _B], bf16)
                if _evac_route(ogp) == "split":
                    # h2 = w + 99*relu(w), w = 0.01*(z2 + b2)
                    wv = tmppool.tile([128, _B], bf16)
                    uv = tmppool.tile([128, _B], bf16)
                    nc.vector.tensor_scalar(
                        out=wv[:], in0=z2p[:], scalar1=cols3_t[:, 256 + ogp:257 + ogp],
                        scalar2=_ALPHA, op0=ALU.add, op1=ALU.mult)
                    nc.vector.tensor_scalar(
                        out=uv[:], in0=wv[:], scalar1=0.0, scalar2=99.0,
                        op0=ALU.max, op1=ALU.mult)
                    nc.gpsimd.tensor_tensor(
                        out=h2[:], in0=wv[:], in1=uv[:], op=ALU.add)
                else:
                    nc.scalar.activation(h2[:], z2p[:], AF.Lrelu,
                                         bias=cols3_t[:, 256 + ogp:257 + ogp],
                                         scale=1.0, alpha=_ALPHA)
                return h2

            # software pipeline: mm4 runs 2 ogs behind, evac 1 og behind, so
            # every emitted instruction's inputs are already (nearly) ready.
            pend_evacs = []  # [(z2, og), ...], evac runs 1 og behind
            pend_mm4 = None  # (h2, og)

            def flush_evacs(cur_og):
                nonlocal pend_mm4
                keep = []
                for z2p, ogp in pend_evacs:
                    if cur_og - ogp >= 1:
                        if pend_mm4 is not None:
                            emit_mm4(*pend_mm4, last=False)
                        h2p = emit_evac(z2p, ogp)
                        pend_mm4 = (h2p, ogp)
                    else:
                        keep.append((z2p, ogp))
                pend_evacs[:] = keep

            for o in range(_OLOC):
                if o == 0:
                    w2_t = w2_t0
                else:
                    w2_t = w2pool.tile([128, 16 * 128], bf16)
                    nc.sync.dma_start(
                        out=w2_t[:],
                        in_=w2blk_d[:, o * 16:(o + 1) * 16].rearrange(
                            "p g m -> p (g m)"),
                    )
                for g in range(16):
                    og = o * 16 + g
                    if o == 0 and 2 <= g < 15:
                        # prefetch next xrep slice during the first o pass
                        gn = g + 1
                        nc.sync.dma_start(out=xrep_t[:, gn * _B:(gn + 1) * _B],
                                          in_=xrep_d[:, gn * _B:(gn + 1) * _B])
                    xg = xrep_t[:, g * _B:(g + 1) * _B]
                    # h1 = t' + 99*relu(t');  t' = 0.01*(W1*x + b1)
                    h1 = h1pool.tile([128, _B], bf16)
                    tp = tmppool.tile([128, _B], bf16)
                    up = tmppool.tile([128, _B], bf16)
                    nc.vector.tensor_scalar(
                        out=tp[:], in0=xg, scalar1=cols3_t[:, og:og + 1],
                        scalar2=cols3_t[:, 128 + og:129 + og],
                        op0=ALU.mult, op1=ALU.add)
                    nc.vector.tensor_scalar(
                        out=up[:], in0=tp[:], scalar1=0.0, scalar2=99.0,
                        op0=ALU.max, op1=ALU.mult)
                    if _h1_route(og) == "dve":
                        nc.vector.tensor_tensor(
                            out=h1[:], in0=tp[:], in1=up[:], op=ALU.add)
                    else:
                        nc.gpsimd.tensor_tensor(
                            out=h1[:], in0=tp[:], in1=up[:], op=ALU.add)
                    z2 = zpool.tile([128, _B], f32)
                    for half in range(2):
                        sl = slice(half * _NHALF, (half + 1) * _NHALF)
                        nc.tensor.matmul(out=z2[:, sl],
                                         lhsT=w2_t[:, g * 128:(g + 1) * 128],
                                         rhs=h1[:, sl], start=True, stop=True)
                    flush_evacs(og)
                    pend_evacs.append((z2, og))
            # drain: flush all but the final og, then MM5 (bias_w*leaky(x) +
            # consts) joins the accumulation so PE's output group never waits
            # on lx/table-load. The last og's evac/mm4/copy/DMA pipeline per
            # PSUM-bank half so half 0 streams out while half 1 is still
            # being evacuated.
            while len(pend_evacs) > 1:
                z2p, ogp = pend_evacs.pop(0)
                if pend_mm4 is not None:
                    emit_mm4(*pend_mm4, last=False)
                h2p = emit_evac(z2p, ogp)
                pend_mm4 = (h2p, ogp)
            emit_mm4(*pend_mm4, last=False)
            for half in range(2):
                sl = slice(half * _NHALF, (half + 1) * _NHALF)
                nc.tensor.matmul(out=outp[:, sl], lhsT=st5_t[:], rhs=lxT_t[:, sl],
                                 start=False, stop=False, skip_group_check=True)
            z2p, ogp = pend_evacs.pop(0)
            h2p = h2pool.tile([128, _B], bf16)
            outs = cpool.tile([8, _B], bf16)
            for half in range(2):
                sl = slice(half * _NHALF, (half + 1) * _NHALF)
                nc.scalar.activation(h2p[:, sl], z2p[:, sl], AF.Lrelu,
                                     bias=cols3_t[:, 256 + ogp:257 + ogp],
                                     scale=1.0, alpha=_ALPHA)
                nc.tensor.matmul(out=outp[:, sl],
                                 lhsT=st4_t[:, ogp * 8:(ogp + 1) * 8],
                                 rhs=h2p[:, sl], start=False, stop=True,
                                 skip_group_check=True)
                if half == 0:
                    nc.vector.tensor_copy(outs[:, sl], outp[:, sl])
                    nc.sync.dma_start(out=out_d[:, sl], in_=outs[:, sl])
                else:
                    nc.scalar.copy(outs[:, sl], outp[:, sl])
                    nc.scalar.dma_start(out=out_d[:, sl], in_=outs[:, sl])

    nc.finalize()
    return nc


def _prepare_inputs(x, W1, b1, W2, b2, W3, b3, layer_w, bias_w):
    f = np.float32
    x = np.asarray(x, f)
    xT = np.ascontiguousarray(x.T)                      # [I, B]
    # xrepb[32j+h, g*B+b] = x[4g+j, b]
    xq = xT.reshape(16, 4, _B).transpose(1, 0, 2)       # [j, g, b]
    xrepb = np.ascontiguousarray(
        np.repeat(xq, _H, axis=0).reshape(128, 16 * _B)).astype(_BF16)
    xt65 = np.concatenate([xT, np.ones((1, _B), f)], 0).astype(_BF16)  # [65, B]

    v = (np.asarray(layer_w, f)[:, :, None] * np.asarray(W3, f))  # [O,I,H]
    w2f = np.asarray(W2, f)

    in_maps = []
    for c in range(_NCORES):
        sl = slice(c * _OLOC, (c + 1) * _OLOC)
        W1c, b1c, b2c = W1[sl], b1[sl], b2[sl]          # [8,64,H]
        W2c = w2f[sl]                                   # [8,64,H,H]
        vc = v[sl]
        lwc, bwc, b3c = layer_w[sl], bias_w[sl], b3[sl]

        # [o, g, j, h] -> partition 32j+h, col o*16+g
        def cols(a):  # a [8, 64, 32] -> [128, 128]
            a = np.asarray(a, f).reshape(_OLOC, 16, 4, _H)
            return np.ascontiguousarray(
                a.transpose(2, 3, 0, 1).reshape(128, 128))

        # packed [w1col | b1col | b2col]; w1/b1 pre-scaled (t' = 0.01*(W1x+b1))
        cols3 = np.ascontiguousarray(np.concatenate(
            [cols(W1c) * _ALPHA, cols(b1c) * _ALPHA, cols(b2c)], axis=1))

        # block-diagonal lhsT: blk[og][32j+h, 32j+k] = W2[o,4g+j,k,h]
        W2t = W2c.transpose(0, 1, 3, 2).reshape(_OLOC, 16, 4, _H, _H)
        w2blk = np.zeros((_OLOC, 16, 128, 128), f)
        for j in range(4):
            w2blk[:, :, 32 * j:32 * j + 32, 32 * j:32 * j + 32] = W2t[:, :, j]
        # -> [p, og, m] layout, bf16
        w2blkb = np.ascontiguousarray(
            w2blk.reshape(128, 128, 128).transpose(1, 0, 2)).astype(_BF16)

        # st4[og][32j+k, o] = v[o,4g+j,k]
        def stack8b(a):
            a = np.asarray(a, f).reshape(_OLOC, 16, 4 * _H)
            out = np.zeros((128, _OLOC * 16, _OLOC), f)
            for o in range(_OLOC):
                for g in range(16):
                    out[:, o * 16 + g, o] = a[o, g]
            return np.ascontiguousarray(out.reshape(128, 128 * _OLOC))

        st4b = stack8b(vc).astype(_BF16)

        st5 = np.zeros((65, _OLOC), f)
        st5[:_I, :] = np.asarray(bwc, f).T              # bias_w[o,i] at row i
        const = (np.asarray(lwc, f) * np.asarray(b3c, f)).sum(1)
        st5[_I, :] = const
        st5b = st5.astype(_BF16)

        in_maps.append({
            "xrepb": xrepb, "xt65": xt65,
            "cols3": cols3,
            "w2blkb": w2blkb, "st4b": st4b, "st5b": st5b,
        })
    return in_maps


def kernel(x, W1, b1, W2, b2, W3, b3, layer_w, bias_w):
    from concourse.bass_utils import run_bass_kernel_spmd

    if "nc" not in _CACHE:
        _CACHE["nc"] = _build_bass()
    nc = _CACHE["nc"]

    in_maps = _prepare_inputs(x, W1, b1, W2, b2, W3, b3, layer_w, bias_w)
    res = run_bass_kernel_spmd(nc, in_maps, list(range(_NCORES))).results

    out = np.empty((_B, _O), np.float32)
    for c in range(_NCORES):
        out[:, c * _OLOC:(c + 1) * _OLOC] = np.asarray(
            res[c]["out"], np.float32).T
    return out


if __name__ == "__main__":
    # quick self-check against a numpy reference
    rng = np.random.default_rng(0)
    f = np.float32
    inputs = {
        "x": rng.standard_normal((_B, _I), f),
        "W1": rng.uniform(-1, 1, (_O, _I, _H)).astype(f),
        "b1": rng.uniform(-1, 1, (_O, _I, _H)).astype(f),
        "W2": rng.uniform(-0.2, 0.2, (_O, _I, _H, _H)).astype(f),
        "b2": rng.uniform(-0.2, 0.2, (_O, _I, _H)).astype(f),
        "W3": rng.uniform(-0.2, 0.2, (_O, _I, _H)).astype(f),
        "b3": rng.uniform(-0.2, 0.2, (_O, _I)).astype(f),
        "layer_w": np.ones((_O, _I), f),
        "bias_w": rng.uniform(-0.1, 0.1, (_O, _I)).astype(f),
    }

    def leaky(a):
        return np.where(a >= 0, a, _ALPHA * a)

    def ref(x, W1, b1, W2, b2, W3, b3, layer_w, bias_w):
        h1 = leaky(x[:, None, :, None] * W1 + b1)
        h2 = leaky(np.einsum("boih,oikh->boik", h1, W2) + b2)
        edge = np.einsum("boih,oih->boi", h2, W3) + b3
        edge = bias_w * leaky(x)[:, None, :] + layer_w * edge
        return edge.sum(axis=2)

    expected = ref(**{k: np.asarray(val, np.float64) for k, val in inputs.items()})
    actual = kernel(**inputs)
    err = np.abs(actual - expected).max() / np.abs(expected).max()
    print("rel err:", err)


# revision 55
# speedup vs baseline: 1.0833x; 1.0023x over previous
"""KAN layer (per-edge tiny MLPs) Trainium2 kernel.

Math (per batch b, output o, input i; H=32 hidden):
  h1 = leaky(x[b,i]*W1[o,i,:] + b1[o,i,:])
  z2 = W2[o,i] @ h1 + b2[o,i]           (per-edge [H,H] matmul)
  h2 = leaky(z2)
  edge = W3[o,i]·h2 + b3[o,i]
  out[b,o] = sum_i (bias_w[o,i]*leaky(x[b,i]) + layer_w[o,i]*edge)

Mapping (8 cores, O sharded, 8 output rows per core), bf16 datapath.
leaky(t) rewritten Pool/DVE-friendly as t' + 99*relu(t') with t' = 0.01*t
(Pool/GPSIMD compiles tensor_scalar + tensor_tensor(add) but not tt(max) or
scalar_tensor_tensor).  Work is split at the *op* level across ACT (1-op
Lrelu evac ~982ns), DVE (tensor_scalar 4x-mode 327ns / tt-add 2x 594ns) and
Pool (flat 853ns/op), targeting ~110us/engine to match PE's 109.7us
(514 bf16 matmuls, cost = out-cols * 0.4167ns).
"""
import sys

sys.path.insert(0, "/opt/trn_rl_repo")

import numpy as np
import ml_dtypes

_B, _I, _O, _H = 1024, 64, 64, 32
_NCORES = 8
_OLOC = _O // _NCORES  # 8 output nodes per core
_ALPHA = 0.01
_NHALF = 512
_BF16 = ml_dtypes.bfloat16

# Pool/GPSIMD cannot read PSUM on real hardware (walrus codegen rejects it).
_POOL_PSUM = False


# --- per-(o,g) routing: balance ACT / DVE / Pool at the op level ---
# Patterns are exclusive (never both on one og) so DVE load stays smooth:
# a split-evac og adds 1519ns to DVE; stacking a DVE h1-add on the same og
# would make a 2.7us burst that stalls the whole ring.
def _h1_route(og):
    # 'split': t',u on DVE; add on Pool.  'dve': all three on DVE.
    return "dve" if og % 10 == 9 else "split"


def _evac_route(og):
    # 'act': 1-op Lrelu on ACT.  'split': w,u on DVE, add on Pool.
    return "split" if og % 10 == 4 else "act"


_CACHE = {}


def _build_bass():
    import concourse.bacc as bacc
    import concourse.mybir as mybir
    from concourse.tile import TileContext

    f32 = mybir.dt.float32
    bf16 = mybir.dt.bfloat16
    AF = mybir.ActivationFunctionType
    ALU = mybir.AluOpType

    nc = bacc.Bacc("TRN2", target_bir_lowering=False, debug=False)

    xrep_d = nc.declare_dram_parameter("xrepb", [128, 16 * _B], bf16, isOutput=False)
    xt65_d = nc.declare_dram_parameter("xt65", [65, _B], bf16, isOutput=False)
    # packed [w1col | b1col | b2col], w1col/b1col pre-scaled by 0.01 on host
    cols3_d = nc.declare_dram_parameter("cols3", [128, 3 * 128], f32, isOutput=False)
    # [p, og, m] layout so the per-o DMA slice is contiguous per partition
    w2blk_d = nc.declare_dram_parameter("w2blkb", [128, 128, 128], bf16, isOutput=False)
    st4_d = nc.declare_dram_parameter("st4b", [128, 128 * 8], bf16, isOutput=False)
    st5_d = nc.declare_dram_parameter("st5b", [65, 8], bf16, isOutput=False)
    out_d = nc.declare_dram_parameter("out", [8, _B], bf16, isOutput=True)

    with TileContext(nc) as tc:
        with tc.tile_pool(name="consts", bufs=1) as cpool, \
             tc.tile_pool(name="w2", bufs=2) as w2pool, \
             tc.tile_pool(name="h1", bufs=6) as h1pool, \
             tc.tile_pool(name="h2", bufs=4) as h2pool, \
             tc.tile_pool(name="tmp", bufs=10) as tmppool, \
             tc.tile_pool(name="zps", bufs=3, space="PSUM") as zpool, \
             tc.tile_pool(name="ops", bufs=1, space="PSUM") as opool:

            # DMA order = priority order: xrep[g0] + packed h1-chain consts
            # first so DVE starts ~2.5us in, then w2[0] for the first matmul,
            # then early xrep slices; st4/st5 (first needed at mm4/drain) last.
            # og-0 ramp: first xrep slice and the first w2 tile arrive in
            # half-chunks so og 0's per-half chain starts ~2us earlier
            xrep_t = cpool.tile([128, 16 * _B], bf16)
            nc.sync.dma_start(out=xrep_t[:, 0:_NHALF], in_=xrep_d[:, 0:_NHALF])
            cols3_t = cpool.tile([128, 3 * 128], f32)
            nc.sync.dma_start(out=cols3_t[:], in_=cols3_d[:])
            w2_t0 = w2pool.tile([128, 16 * 128], bf16)
            nc.sync.dma_start(
                out=w2_t0[:, 0:8 * 128],
                in_=w2blk_d[:, 0:8].rearrange("p g m -> p (g m)"))
            nc.sync.dma_start(out=xrep_t[:, _NHALF:_B], in_=xrep_d[:, _NHALF:_B])
            nc.sync.dma_start(
                out=w2_t0[:, 8 * 128:16 * 128],
                in_=w2blk_d[:, 8:16].rearrange("p g m -> p (g m)"))
            nc.sync.dma_start(out=xrep_t[:, _B:2 * _B], in_=xrep_d[:, _B:2 * _B])
            st4_t = cpool.tile([128, 128 * 8], bf16)
            nc.sync.dma_start(out=st4_t[:], in_=st4_d[:])
            nc.sync.dma_start(out=xrep_t[:, 2 * _B:3 * _B],
                              in_=xrep_d[:, 2 * _B:3 * _B])
            # xt65/st5 feed only the drain-time MM5; ride the startup-idle
            # ACT HWDGE queue to keep SP free for xrep slices and w2 blocks
            xt65_t = cpool.tile([65, _B], bf16)
            nc.scalar.dma_start(out=xt65_t[:], in_=xt65_d[:])
            st5_t = cpool.tile([65, 8], bf16)
            nc.scalar.dma_start(out=st5_t[:], in_=st5_d[:])

            lxT_t = cpool.tile([65, _B], bf16)
            nc.scalar.activation(lxT_t[:], xt65_t[:], AF.Lrelu,
                                 bias=0.0, scale=1.0, alpha=_ALPHA)

            outp = opool.tile([8, _B], f32)
            acc_started = [False]

            def emit_mm4(h2_prev, og_prev, last):
                for half in range(2):
                    sl = slice(half * _NHALF, (half + 1) * _NHALF)
                    nc.tensor.matmul(out=outp[:, sl],
                                     lhsT=st4_t[:, og_prev * 8:(og_prev + 1) * 8],
                                     rhs=h2_prev[:, sl],
                                     start=not acc_started[0], stop=last,
                                     skip_group_check=True)
                acc_started[0] = True

            def emit_evac(z2p, ogp):
                h2 = h2pool.tile([128, _B], bf16)
                if _evac_route(ogp) == "split":
                    # h2 = w + 99*relu(w), w = 0.01*(z2 + b2)
                    wv = tmppool.tile([128, _B], bf16)
                    uv = tmppool.tile([128, _B], bf16)
                    nc.vector.tensor_scalar(
                        out=wv[:], in0=z2p[:], scalar1=cols3_t[:, 256 + ogp:257 + ogp],
                        scalar2=_ALPHA, op0=ALU.add, op1=ALU.mult)
                    nc.vector.tensor_scalar(
                        out=uv[:], in0=wv[:], scalar1=0.0, scalar2=99.0,
                        op0=ALU.max, op1=ALU.mult)
                    nc.gpsimd.tensor_tensor(
                        out=h2[:], in0=wv[:], in1=uv[:], op=ALU.add)
                else:
                    nc.scalar.activation(h2[:], z2p[:], AF.Lrelu,
                                         bias=cols3_t[:, 256 + ogp:257 + ogp],
                                         scale=1.0, alpha=_ALPHA)
                return h2

            # software pipeline: mm4 runs 2 ogs behind, evac 1 og behind, so
            # every emitted instruction's inputs are already (nearly) ready.
            pend_evacs = []  # [(z2, og), ...], evac runs 1 og behind
            pend_mm4 = None  # (h2, og)

            def flush_evacs(cur_og):
                nonlocal pend_mm4
                keep = []
                for z2p, ogp in pend_evacs:
                    if cur_og - ogp >= 1:
                        if pend_mm4 is not None:
                            emit_mm4(*pend_mm4, last=False)
                        h2p = emit_evac(z2p, ogp)
                        pend_mm4 = (h2p, ogp)
                    else:
                        keep.append((z2p, ogp))
                pend_evacs[:] = keep

            for o in range(_OLOC):
                if o == 0:
                    w2_t = w2_t0
                else:
                    w2_t = w2pool.tile([128, 16 * 128], bf16)
                    nc.sync.dma_start(
                        out=w2_t[:],
                        in_=w2blk_d[:, o * 16:(o + 1) * 16].rearrange(
                            "p g m -> p (g m)"),
                    )
                for g in range(16):
                    og = o * 16 + g
                    if o == 0 and 2 <= g < 15:
                        # prefetch next xrep slice during the first o pass
                        gn = g + 1
                        nc.sync.dma_start(out=xrep_t[:, gn * _B:(gn + 1) * _B],
                                          in_=xrep_d[:, gn * _B:(gn + 1) * _B])
                    xg = xrep_t[:, g * _B:(g + 1) * _B]
                    # h1 = t' + 99*relu(t');  t' = 0.01*(W1*x + b1)
                    h1 = h1pool.tile([128, _B], bf16)
                    tp = tmppool.tile([128, _B], bf16)
                    up = tmppool.tile([128, _B], bf16)
                    # og 0 runs per half so its first MM2 launches off the
                    # half-chunk DMAs ~1.6us earlier
                    for hs in ([slice(0, _NHALF), slice(_NHALF, _B)]
                               if og == 0 else [slice(0, _B)]):
                        nc.vector.tensor_scalar(
                            out=tp[:, hs], in0=xg[:, hs],
                            scalar1=cols3_t[:, og:og + 1],
                            scalar2=cols3_t[:, 128 + og:129 + og],
                            op0=ALU.mult, op1=ALU.add)
                        nc.vector.tensor_scalar(
                            out=up[:, hs], in0=tp[:, hs], scalar1=0.0,
                            scalar2=99.0, op0=ALU.max, op1=ALU.mult)
                        if _h1_route(og) == "dve":
                            nc.vector.tensor_tensor(
                                out=h1[:, hs], in0=tp[:, hs], in1=up[:, hs],
                                op=ALU.add)
                        else:
                            nc.gpsimd.tensor_tensor(
                                out=h1[:, hs], in0=tp[:, hs], in1=up[:, hs],
                                op=ALU.add)
                    z2 = zpool.tile([128, _B], f32)
                    for half in range(2):
                        sl = slice(half * _NHALF, (half + 1) * _NHALF)
                        nc.tensor.matmul(out=z2[:, sl],
                                         lhsT=w2_t[:, g * 128:(g + 1) * 128],
                                         rhs=h1[:, sl], start=True, stop=True)
                    flush_evacs(og)
                    pend_evacs.append((z2, og))
            # drain: flush all but the final og, then MM5 (bias_w*leaky(x) +
            # consts) joins the accumulation so PE's output group never waits
            # on lx/table-load. The last og's evac/mm4/copy/DMA pipeline per
            # PSUM-bank half so half 0 streams out while half 1 is still
            # being evacuated.
            while len(pend_evacs) > 1:
                z2p, ogp = pend_evacs.pop(0)
                if pend_mm4 is not None:
                    emit_mm4(*pend_mm4, last=False)
                h2p = emit_evac(z2p, ogp)
                pend_mm4 = (h2p, ogp)
            emit_mm4(*pend_mm4, last=False)
            for half in range(2):
                sl = slice(half * _NHALF, (half + 1) * _NHALF)
                nc.tensor.matmul(out=outp[:, sl], lhsT=st5_t[:], rhs=lxT_t[:, sl],
                                 start=False, stop=False, skip_group_check=True)
            z2p, ogp = pend_evacs.pop(0)
            h2p = h2pool.tile([128, _B], bf16)
            outs = cpool.tile([8, _B], bf16)
            for half in range(2):
                sl = slice(half * _NHALF, (half + 1) * _NHALF)
                nc.scalar.activation(h2p[:, sl], z2p[:, sl], AF.Lrelu,
                                     bias=cols3_t[:, 256 + ogp:257 + ogp],
                                     scale=1.0, alpha=_ALPHA)
                nc.tensor.matmul(out=outp[:, sl],
                                 lhsT=st4_t[:, ogp * 8:(ogp + 1) * 8],
                                 rhs=h2p[:, sl], start=False, stop=True,
                                 skip_group_check=True)
                if half == 0:
                    nc.vector.tensor_copy(outs[:, sl], outp[:, sl])
                    nc.sync.dma_start(out=out_d[:, sl], in_=outs[:, sl])
                else:
                    nc.scalar.copy(outs[:, sl], outp[:, sl])
                    nc.scalar.dma_start(out=out_d[:, sl], in_=outs[:, sl])

    nc.finalize()
    return nc


def _prepare_inputs(x, W1, b1, W2, b2, W3, b3, layer_w, bias_w):
    f = np.float32
    x = np.asarray(x, f)
    xT = np.ascontiguousarray(x.T)                      # [I, B]
    # xrepb[32j+h, g*B+b] = x[4g+j, b]
    xq = xT.reshape(16, 4, _B).transpose(1, 0, 2)       # [j, g, b]
    xrepb = np.ascontiguousarray(
        np.repeat(xq, _H, axis=0).reshape(128, 16 * _B)).astype(_BF16)
    xt65 = np.concatenate([xT, np.ones((1, _B), f)], 0).astype(_BF16)  # [65, B]

    v = (np.asarray(layer_w, f)[:, :, None] * np.asarray(W3, f))  # [O,I,H]
    w2f = np.asarray(W2, f)

    in_maps = []
    for c in range(_NCORES):
        sl = slice(c * _OLOC, (c + 1) * _OLOC)
        W1c, b1c, b2c = W1[sl], b1[sl], b2[sl]          # [8,64,H]
        W2c = w2f[sl]                                   # [8,64,H,H]
        vc = v[sl]
        lwc, bwc, b3c = layer_w[sl], bias_w[sl], b3[sl]

        # [o, g, j, h] -> partition 32j+h, col o*16+g
        def cols(a):  # a [8, 64, 32] -> [128, 128]
            a = np.asarray(a, f).reshape(_OLOC, 16, 4, _H)
            return np.ascontiguousarray(
                a.transpose(2, 3, 0, 1).reshape(128, 128))

        # packed [w1col | b1col | b2col]; w1/b1 pre-scaled (t' = 0.01*(W1x+b1))
        cols3 = np.ascontiguousarray(np.concatenate(
            [cols(W1c) * _ALPHA, cols(b1c) * _ALPHA, cols(b2c)], axis=1))

        # block-diagonal lhsT: blk[og][32j+h, 32j+k] = W2[o,4g+j,k,h]
        W2t = W2c.transpose(0, 1, 3, 2).reshape(_OLOC, 16, 4, _H, _H)
        w2blk = np.zeros((_OLOC, 16, 128, 128), f)
        for j in range(4):
            w2blk[:, :, 32 * j:32 * j + 32, 32 * j:32 * j + 32] = W2t[:, :, j]
        # -> [p, og, m] layout, bf16
        w2blkb = np.ascontiguousarray(
            w2blk.reshape(128, 128, 128).transpose(1, 0, 2)).astype(_BF16)

        # st4[og][32j+k, o] = v[o,4g+j,k]
        def stack8b(a):
            a = np.asarray(a, f).reshape(_OLOC, 16, 4 * _H)
            out = np.zeros((128, _OLOC * 16, _OLOC), f)
            for o in range(_OLOC):
                for g in range(16):
                    out[:, o * 16 + g, o] = a[o, g]
            return np.ascontiguousarray(out.reshape(128, 128 * _OLOC))

        st4b = stack8b(vc).astype(_BF16)

        st5 = np.zeros((65, _OLOC), f)
        st5[:_I, :] = np.asarray(bwc, f).T              # bias_w[o,i] at row i
        const = (np.asarray(lwc, f) * np.asarray(b3c, f)).sum(1)
        st5[_I, :] = const
        st5b = st5.astype(_BF16)

        in_maps.append({
            "xrepb": xrepb, "xt65": xt65,
            "cols3": cols3,
            "w2blkb": w2blkb, "st4b": st4b, "st5b": st5b,
        })
    return in_maps


def kernel(x, W1, b1, W2, b2, W3, b3, layer_w, bias_w):
    from concourse.bass_utils import run_bass_kernel_spmd

    if "nc" not in _CACHE:
        _CACHE["nc"] = _build_bass()
    nc = _CACHE["nc"]

    in_maps = _prepare_inputs(x, W1, b1, W2, b2, W3, b3, layer_w, bias_w)
    res = run_bass_kernel_spmd(nc, in_maps, list(range(_NCORES))).results

    out = np.empty((_B, _O), np.float32)
    for c in range(_NCORES):
        out[:, c * _OLOC:(c + 1) * _OLOC] = np.asarray(
            res[c]["out"], np.float32).T
    return out


if __name__ == "__main__":
    # quick self-check against a numpy reference
    rng = np.random.default_rng(0)
    f = np.float32
    inputs = {
        "x": rng.standard_normal((_B, _I), f),
        "W1": rng.uniform(-1, 1, (_O, _I, _H)).astype(f),
        "b1": rng.uniform(-1, 1, (_O, _I, _H)).astype(f),
        "W2": rng.uniform(-0.2, 0.2, (_O, _I, _H, _H)).astype(f),
        "b2": rng.uniform(-0.2, 0.2, (_O, _I, _H)).astype(f),
        "W3": rng.uniform(-0.2, 0.2, (_O, _I, _H)).astype(f),
        "b3": rng.uniform(-0.2, 0.2, (_O, _I)).astype(f),
        "layer_w": np.ones((_O, _I), f),
        "bias_w": rng.uniform(-0.1, 0.1, (_O, _I)).astype(f),
    }

    def leaky(a):
        return np.where(a >= 0, a, _ALPHA * a)

    def ref(x, W1, b1, W2, b2, W3, b3, layer_w, bias_w):
        h1 = leaky(x[:, None, :, None] * W1 + b1)
        h2 = leaky(np.einsum("boih,oikh->boik", h1, W2) + b2)
        edge = np.einsum("boih,oih->boi", h2, W3) + b3
        edge = bias_w * leaky(x)[:, None, :] + layer_w * edge
        return edge.sum(axis=2)

    expected = ref(**{k: np.asarray(val, np.float64) for k, val in inputs.items()})
    actual = kernel(**inputs)
    err = np.abs(actual - expected).max() / np.abs(expected).max()
    print("rel err:", err)


# revision 56
# speedup vs baseline: 1.0853x; 1.0018x over previous
"""KAN layer (per-edge tiny MLPs) Trainium2 kernel.

Math (per batch b, output o, input i; H=32 hidden):
  h1 = leaky(x[b,i]*W1[o,i,:] + b1[o,i,:])
  z2 = W2[o,i] @ h1 + b2[o,i]           (per-edge [H,H] matmul)
  h2 = leaky(z2)
  edge = W3[o,i]·h2 + b3[o,i]
  out[b,o] = sum_i (bias_w[o,i]*leaky(x[b,i]) + layer_w[o,i]*edge)

Mapping (8 cores, O sharded, 8 output rows per core), bf16 datapath.
leaky(t) rewritten Pool/DVE-friendly as t' + 99*relu(t') with t' = 0.01*t
(Pool/GPSIMD compiles tensor_scalar + tensor_tensor(add) but not tt(max) or
scalar_tensor_tensor).  Work is split at the *op* level across ACT (1-op
Lrelu evac ~982ns), DVE (tensor_scalar 4x-mode 327ns / tt-add 2x 594ns) and
Pool (flat 853ns/op), targeting ~110us/engine to match PE's 109.7us
(514 bf16 matmuls, cost = out-cols * 0.4167ns).
"""
import sys

sys.path.insert(0, "/opt/trn_rl_repo")

import numpy as np
import ml_dtypes

_B, _I, _O, _H = 1024, 64, 64, 32
_NCORES = 8
_OLOC = _O // _NCORES  # 8 output nodes per core
_ALPHA = 0.01
_NHALF = 512
_BF16 = ml_dtypes.bfloat16

# Pool/GPSIMD cannot read PSUM on real hardware (walrus codegen rejects it).
_POOL_PSUM = False


# --- per-(o,g) routing: balance ACT / DVE / Pool at the op level ---
# Patterns are exclusive (never both on one og) so DVE load stays smooth:
# a split-evac og adds 1519ns to DVE; stacking a DVE h1-add on the same og
# would make a 2.7us burst that stalls the whole ring.
def _h1_route(og):
    # 'split': t',u on DVE; add on Pool.  'dve': all three on DVE.
    return "dve" if og % 10 == 9 else "split"


def _evac_route(og):
    # 'act': 1-op Lrelu on ACT.  'split': w,u on DVE, add on Pool.
    return "split" if og % 10 == 4 else "act"


_CACHE = {}


def _build_bass():
    import concourse.bacc as bacc
    import concourse.mybir as mybir
    from concourse.tile import TileContext

    f32 = mybir.dt.float32
    bf16 = mybir.dt.bfloat16
    AF = mybir.ActivationFunctionType
    ALU = mybir.AluOpType

    nc = bacc.Bacc("TRN2", target_bir_lowering=False, debug=False)

    xrep_d = nc.declare_dram_parameter("xrepb", [128, 16 * _B], bf16, isOutput=False)
    xt65_d = nc.declare_dram_parameter("xt65", [65, _B], bf16, isOutput=False)
    # packed [w1col | b1col | b2col], w1col/b1col pre-scaled by 0.01 on host
    cols3_d = nc.declare_dram_parameter("cols3", [128, 3 * 128], f32, isOutput=False)
    # [p, og, m] layout so the per-o DMA slice is contiguous per partition
    w2blk_d = nc.declare_dram_parameter("w2blkb", [128, 128, 128], bf16, isOutput=False)
    st4_d = nc.declare_dram_parameter("st4b", [128, 128 * 8], bf16, isOutput=False)
    st5_d = nc.declare_dram_parameter("st5b", [65, 8], bf16, isOutput=False)
    out_d = nc.declare_dram_parameter("out", [8, _B], bf16, isOutput=True)

    with TileContext(nc) as tc:
        with tc.tile_pool(name="consts", bufs=1) as cpool, \
             tc.tile_pool(name="w2", bufs=2) as w2pool, \
             tc.tile_pool(name="h1", bufs=6) as h1pool, \
             tc.tile_pool(name="h2", bufs=4) as h2pool, \
             tc.tile_pool(name="tmp", bufs=10) as tmppool, \
             tc.tile_pool(name="zps", bufs=3, space="PSUM") as zpool, \
             tc.tile_pool(name="ops", bufs=1, space="PSUM") as opool:

            # DMA order = priority order: xrep[g0] + packed h1-chain consts
            # first so DVE starts ~2.5us in, then w2[0] for the first matmul,
            # then early xrep slices; st4/st5 (first needed at mm4/drain) last.
            # og-0 ramp: first xrep slice and the first w2 tile arrive in
            # half-chunks so og 0's per-half chain starts ~2us earlier
            xrep_t = cpool.tile([128, 16 * _B], bf16)
            nc.sync.dma_start(out=xrep_t[:, 0:_NHALF], in_=xrep_d[:, 0:_NHALF])
            cols3_t = cpool.tile([128, 3 * 128], f32)
            nc.sync.dma_start(out=cols3_t[:], in_=cols3_d[:])
            w2_t0 = w2pool.tile([128, 16 * 128], bf16)
            nc.sync.dma_start(
                out=w2_t0[:, 0:8 * 128],
                in_=w2blk_d[:, 0:8].rearrange("p g m -> p (g m)"))
            nc.sync.dma_start(out=xrep_t[:, _NHALF:_B], in_=xrep_d[:, _NHALF:_B])
            nc.sync.dma_start(
                out=w2_t0[:, 8 * 128:16 * 128],
                in_=w2blk_d[:, 8:16].rearrange("p g m -> p (g m)"))
            nc.sync.dma_start(out=xrep_t[:, _B:2 * _B], in_=xrep_d[:, _B:2 * _B])
            st4_t = cpool.tile([128, 128 * 8], bf16)
            nc.sync.dma_start(out=st4_t[:], in_=st4_d[:])
            nc.sync.dma_start(out=xrep_t[:, 2 * _B:3 * _B],
                              in_=xrep_d[:, 2 * _B:3 * _B])
            # xt65/st5 feed only the drain-time MM5; ride the startup-idle
            # ACT HWDGE queue to keep SP free for xrep slices and w2 blocks
            xt65_t = cpool.tile([65, _B], bf16)
            nc.scalar.dma_start(out=xt65_t[:], in_=xt65_d[:])
            st5_t = cpool.tile([65, 8], bf16)
            nc.scalar.dma_start(out=st5_t[:], in_=st5_d[:])

            lxT_t = cpool.tile([65, _B], bf16)
            nc.scalar.activation(lxT_t[:], xt65_t[:], AF.Lrelu,
                                 bias=0.0, scale=1.0, alpha=_ALPHA)

            outp = opool.tile([8, _B], f32)
            acc_started = [False]

            def emit_mm4(h2_prev, og_prev, last):
                for half in range(2):
                    sl = slice(half * _NHALF, (half + 1) * _NHALF)
                    nc.tensor.matmul(out=outp[:, sl],
                                     lhsT=st4_t[:, og_prev * 8:(og_prev + 1) * 8],
                                     rhs=h2_prev[:, sl],
                                     start=not acc_started[0], stop=last,
                                     skip_group_check=True)
                acc_started[0] = True

            def emit_evac(z2p, ogp):
                h2 = h2pool.tile([128, _B], bf16)
                if _evac_route(ogp) == "split":
                    # h2 = w + 99*relu(w), w = 0.01*(z2 + b2)
                    wv = tmppool.tile([128, _B], bf16)
                    uv = tmppool.tile([128, _B], bf16)
                    nc.vector.tensor_scalar(
                        out=wv[:], in0=z2p[:], scalar1=cols3_t[:, 256 + ogp:257 + ogp],
                        scalar2=_ALPHA, op0=ALU.add, op1=ALU.mult)
                    nc.vector.tensor_scalar(
                        out=uv[:], in0=wv[:], scalar1=0.0, scalar2=99.0,
                        op0=ALU.max, op1=ALU.mult)
                    nc.gpsimd.tensor_tensor(
                        out=h2[:], in0=wv[:], in1=uv[:], op=ALU.add)
                else:
                    nc.scalar.activation(h2[:], z2p[:], AF.Lrelu,
                                         bias=cols3_t[:, 256 + ogp:257 + ogp],
                                         scale=1.0, alpha=_ALPHA)
                return h2

            # software pipeline: mm4 runs 2 ogs behind, evac 1 og behind, so
            # every emitted instruction's inputs are already (nearly) ready.
            pend_evacs = []  # [(z2, og), ...], evac runs 1 og behind
            pend_mm4 = None  # (h2, og)

            def flush_evacs(cur_og):
                nonlocal pend_mm4
                keep = []
                for z2p, ogp in pend_evacs:
                    if cur_og - ogp >= 1:
                        if pend_mm4 is not None:
                            emit_mm4(*pend_mm4, last=False)
                        h2p = emit_evac(z2p, ogp)
                        pend_mm4 = (h2p, ogp)
                    else:
                        keep.append((z2p, ogp))
                pend_evacs[:] = keep

            for o in range(_OLOC):
                if o == 0:
                    w2_t = w2_t0
                else:
                    w2_t = w2pool.tile([128, 16 * 128], bf16)
                    nc.sync.dma_start(
                        out=w2_t[:],
                        in_=w2blk_d[:, o * 16:(o + 1) * 16].rearrange(
                            "p g m -> p (g m)"),
                    )
                for g in range(16):
                    og = o * 16 + g
                    if o == 0 and 2 <= g < 15:
                        # prefetch next xrep slice during the first o pass
                        gn = g + 1
                        nc.sync.dma_start(out=xrep_t[:, gn * _B:(gn + 1) * _B],
                                          in_=xrep_d[:, gn * _B:(gn + 1) * _B])
                    xg = xrep_t[:, g * _B:(g + 1) * _B]
                    # h1 = t' + 99*relu(t');  t' = 0.01*(W1*x + b1)
                    h1 = h1pool.tile([128, _B], bf16)
                    tp = tmppool.tile([128, _B], bf16)
                    up = tmppool.tile([128, _B], bf16)
                    # og 0 runs per half so its first MM2 launches off the
                    # half-chunk DMAs ~1.6us earlier
                    for hs in ([slice(0, _NHALF), slice(_NHALF, _B)]
                               if og <= 1 else [slice(0, _B)]):
                        nc.vector.tensor_scalar(
                            out=tp[:, hs], in0=xg[:, hs],
                            scalar1=cols3_t[:, og:og + 1],
                            scalar2=cols3_t[:, 128 + og:129 + og],
                            op0=ALU.mult, op1=ALU.add)
                        nc.vector.tensor_scalar(
                            out=up[:, hs], in0=tp[:, hs], scalar1=0.0,
                            scalar2=99.0, op0=ALU.max, op1=ALU.mult)
                        if _h1_route(og) == "dve":
                            nc.vector.tensor_tensor(
                                out=h1[:, hs], in0=tp[:, hs], in1=up[:, hs],
                                op=ALU.add)
                        else:
                            nc.gpsimd.tensor_tensor(
                                out=h1[:, hs], in0=tp[:, hs], in1=up[:, hs],
                                op=ALU.add)
                    z2 = zpool.tile([128, _B], f32)
                    for half in range(2):
                        sl = slice(half * _NHALF, (half + 1) * _NHALF)
                        nc.tensor.matmul(out=z2[:, sl],
                                         lhsT=w2_t[:, g * 128:(g + 1) * 128],
                                         rhs=h1[:, sl], start=True, stop=True)
                    flush_evacs(og)
                    pend_evacs.append((z2, og))
            # drain: flush all but the final og, then MM5 (bias_w*leaky(x) +
            # consts) joins the accumulation so PE's output group never waits
            # on lx/table-load. The last og's evac/mm4/copy/DMA pipeline per
            # PSUM-bank half so half 0 streams out while half 1 is still
            # being evacuated.
            while len(pend_evacs) > 1:
                z2p, ogp = pend_evacs.pop(0)
                if pend_mm4 is not None:
                    emit_mm4(*pend_mm4, last=False)
                h2p = emit_evac(z2p, ogp)
                pend_mm4 = (h2p, ogp)
            emit_mm4(*pend_mm4, last=False)
            for half in range(2):
                sl = slice(half * _NHALF, (half + 1) * _NHALF)
                nc.tensor.matmul(out=outp[:, sl], lhsT=st5_t[:], rhs=lxT_t[:, sl],
                                 start=False, stop=False, skip_group_check=True)
            z2p, ogp = pend_evacs.pop(0)
            h2p = h2pool.tile([128, _B], bf16)
            outs = cpool.tile([8, _B], bf16)
            for half in range(2):
                sl = slice(half * _NHALF, (half + 1) * _NHALF)
                nc.scalar.activation(h2p[:, sl], z2p[:, sl], AF.Lrelu,
                                     bias=cols3_t[:, 256 + ogp:257 + ogp],
                                     scale=1.0, alpha=_ALPHA)
                nc.tensor.matmul(out=outp[:, sl],
                                 lhsT=st4_t[:, ogp * 8:(ogp + 1) * 8],
                                 rhs=h2p[:, sl], start=False, stop=True,
                                 skip_group_check=True)
                if half == 0:
                    nc.vector.tensor_copy(outs[:, sl], outp[:, sl])
                    nc.sync.dma_start(out=out_d[:, sl], in_=outs[:, sl])
                else:
                    nc.scalar.copy(outs[:, sl], outp[:, sl])
                    nc.scalar.dma_start(out=out_d[:, sl], in_=outs[:, sl])

    nc.finalize()
    return nc


def _prepare_inputs(x, W1, b1, W2, b2, W3, b3, layer_w, bias_w):
    f = np.float32
    x = np.asarray(x, f)
    xT = np.ascontiguousarray(x.T)                      # [I, B]
    # xrepb[32j+h, g*B+b] = x[4g+j, b]
    xq = xT.reshape(16, 4, _B).transpose(1, 0, 2)       # [j, g, b]
    xrepb = np.ascontiguousarray(
        np.repeat(xq, _H, axis=0).reshape(128, 16 * _B)).astype(_BF16)
    xt65 = np.concatenate([xT, np.ones((1, _B), f)], 0).astype(_BF16)  # [65, B]

    v = (np.asarray(layer_w, f)[:, :, None] * np.asarray(W3, f))  # [O,I,H]
    w2f = np.asarray(W2, f)

    in_maps = []
    for c in range(_NCORES):
        sl = slice(c * _OLOC, (c + 1) * _OLOC)
        W1c, b1c, b2c = W1[sl], b1[sl], b2[sl]          # [8,64,H]
        W2c = w2f[sl]                                   # [8,64,H,H]
        vc = v[sl]
        lwc, bwc, b3c = layer_w[sl], bias_w[sl], b3[sl]

        # [o, g, j, h] -> partition 32j+h, col o*16+g
        def cols(a):  # a [8, 64, 32] -> [128, 128]
            a = np.asarray(a, f).reshape(_OLOC, 16, 4, _H)
            return np.ascontiguousarray(
                a.transpose(2, 3, 0, 1).reshape(128, 128))

        # packed [w1col | b1col | b2col]; w1/b1 pre-scaled (t' = 0.01*(W1x+b1))
        cols3 = np.ascontiguousarray(np.concatenate(
            [cols(W1c) * _ALPHA, cols(b1c) * _ALPHA, cols(b2c)], axis=1))

        # block-diagonal lhsT: blk[og][32j+h, 32j+k] = W2[o,4g+j,k,h]
        W2t = W2c.transpose(0, 1, 3, 2).reshape(_OLOC, 16, 4, _H, _H)
        w2blk = np.zeros((_OLOC, 16, 128, 128), f)
        for j in range(4):
            w2blk[:, :, 32 * j:32 * j + 32, 32 * j:32 * j + 32] = W2t[:, :, j]
        # -> [p, og, m] layout, bf16
        w2blkb = np.ascontiguousarray(
            w2blk.reshape(128, 128, 128).transpose(1, 0, 2)).astype(_BF16)

        # st4[og][32j+k, o] = v[o,4g+j,k]
        def stack8b(a):
            a = np.asarray(a, f).reshape(_OLOC, 16, 4 * _H)
            out = np.zeros((128, _OLOC * 16, _OLOC), f)
            for o in range(_OLOC):
                for g in range(16):
                    out[:, o * 16 + g, o] = a[o, g]
            return np.ascontiguousarray(out.reshape(128, 128 * _OLOC))

        st4b = stack8b(vc).astype(_BF16)

        st5 = np.zeros((65, _OLOC), f)
        st5[:_I, :] = np.asarray(bwc, f).T              # bias_w[o,i] at row i
        const = (np.asarray(lwc, f) * np.asarray(b3c, f)).sum(1)
        st5[_I, :] = const
        st5b = st5.astype(_BF16)

        in_maps.append({
            "xrepb": xrepb, "xt65": xt65,
            "cols3": cols3,
            "w2blkb": w2blkb, "st4b": st4b, "st5b": st5b,
        })
    return in_maps


def kernel(x, W1, b1, W2, b2, W3, b3, layer_w, bias_w):
    from concourse.bass_utils import run_bass_kernel_spmd

    if "nc" not in _CACHE:
        _CACHE["nc"] = _build_bass()
    nc = _CACHE["nc"]

    in_maps = _prepare_inputs(x, W1, b1, W2, b2, W3, b3, layer_w, bias_w)
    res = run_bass_kernel_spmd(nc, in_maps, list(range(_NCORES))).results

    out = np.empty((_B, _O), np.float32)
    for c in range(_NCORES):
        out[:, c * _OLOC:(c + 1) * _OLOC] = np.asarray(
            res[c]["out"], np.float32).T
    return out


if __name__ == "__main__":
    # quick self-check against a numpy reference
    rng = np.random.default_rng(0)
    f = np.float32
    inputs = {
        "x": rng.standard_normal((_B, _I), f),
        "W1": rng.uniform(-1, 1, (_O, _I, _H)).astype(f),
        "b1": rng.uniform(-1, 1, (_O, _I, _H)).astype(f),
        "W2": rng.uniform(-0.2, 0.2, (_O, _I, _H, _H)).astype(f),
        "b2": rng.uniform(-0.2, 0.2, (_O, _I, _H)).astype(f),
        "W3": rng.uniform(-0.2, 0.2, (_O, _I, _H)).astype(f),
        "b3": rng.uniform(-0.2, 0.2, (_O, _I)).astype(f),
        "layer_w": np.ones((_O, _I), f),
        "bias_w": rng.uniform(-0.1, 0.1, (_O, _I)).astype(f),
    }

    def leaky(a):
        return np.where(a >= 0, a, _ALPHA * a)

    def ref(x, W1, b1, W2, b2, W3, b3, layer_w, bias_w):
        h1 = leaky(x[:, None, :, None] * W1 + b1)
        h2 = leaky(np.einsum("boih,oikh->boik", h1, W2) + b2)
        edge = np.einsum("boih,oih->boi", h2, W3) + b3
        edge = bias_w * leaky(x)[:, None, :] + layer_w * edge
        return edge.sum(axis=2)

    expected = ref(**{k: np.asarray(val, np.float64) for k, val in inputs.items()})
    actual = kernel(**inputs)
    err = np.abs(actual - expected).max() / np.abs(expected).max()
    print("rel err:", err)
